# revision 6
# baseline (speedup 1.0000x reference)
"""Trainium2 Bass kernel for nn_Decoder (dense CNN decoder, 8 NeuronCores).

Sharding: data-parallel over batch (B=32 -> 4 samples/core). BatchNorm batch
stats are synced exactly via small AllGather collectives (sum/sumsq per
channel). Convs run as bf16 TensorE matmuls with f32 PSUM accumulation;
dilated k=3 convs are 3 shifted 1x1 matmuls accumulated in PSUM. All weight
transposes (lhsT layouts, fc_w.T) are prepared host-side. Conv biases feeding
BatchNorm cancel exactly (BN shift invariance) and are dropped.

Self-contained: hardcodes all shapes; reads nothing from /root/problem.
"""
import sys

sys.path.insert(0, "/opt/trn_rl_repo")

from contextlib import ExitStack

import numpy as np
import ml_dtypes

import concourse.bass as bass
import concourse.tile as tile
from concourse import bacc, mybir
from concourse.bass_utils import run_bass_kernel_spmd
from concourse.masks import make_identity

N_CORES = 8
B = 32
S = B // N_CORES          # samples per core
T = 256
L = T + 1                 # 257 positions incl. init token
VOC = 12000
EMB = 512
NF = 512
LATENT = 128
DILS = [1, 2, 4, 8, 16, 1, 2, 4, 8, 16]
EPS = 1e-5
CH = NF // 128            # 4 channel chunks

BF16 = mybir.dt.bfloat16
F32 = mybir.dt.float32
F32R = mybir.dt.float32r
I32 = mybir.dt.int32
RELU = mybir.ActivationFunctionType.Relu
IDENT = mybir.ActivationFunctionType.Identity
SQRT = mybir.ActivationFunctionType.Sqrt
ADD = mybir.AluOpType.add
SUB = mybir.AluOpType.subtract
MULT = mybir.AluOpType.mult

_cache = {}


def _build(fc_bias_nonzero, dump=None):
    """dump: None | 'init' | int k (dump x after block k) -> extra output 'dbg'."""
    nc = bacc.Bacc("TRN2", target_bir_lowering=False, debug=False,
                   num_devices=N_CORES)

    def inp(name, shape, dt):
        return nc.dram_tensor(name, shape, dt, kind="ExternalInput").ap()

    helper = inp("helper", [S * T, 1], I32)
    latT = inp("latT", [LATENT, S], F32R)
    emb = inp("emb", [VOC, EMB], F32)
    ti_w1T = inp("ti_w1T", [LATENT, EMB], F32R)
    ti_w2T = inp("ti_w2T", [EMB, EMB], F32R)
    ti_b1 = inp("ti_b1", [EMB], F32)
    ti_b2 = inp("ti_b2", [EMB], F32)
    w1iT = inp("w1iT", [EMB, NF], F32R)
    w2iT = inp("w2iT", [EMB, NF], F32R)
    w12T = inp("w12T", [EMB, NF], F32R)
    init_b = inp("init_b", [NF], F32)
    wconv = inp("wconv", [10, 5, NF, NF], F32R)
    bng = inp("bng", [10, 3, NF], F32)
    bnb = inp("bnb", [10, 3, NF], F32)
    fc_wT = inp("fc_wT", [NF, VOC], BF16)
    fc_b = inp("fc_b", [1, VOC], BF16)
    out = nc.dram_tensor("out", [S, T, VOC], F32, kind="ExternalOutput").ap()
    dbg = None
    if dump is not None:
        dbg = nc.dram_tensor("dbg", [NF, S * L], F32, kind="ExternalOutput").ap()

    def chvec(ap1d):
        # [512] dram vector -> [128, 4] AP (partition p, chunk c) = v[c*128+p]
        return ap1d.rearrange("(c p) -> p c", p=128)

    with tile.TileContext(nc) as tc, ExitStack() as es:
        const = es.enter_context(tc.tile_pool(name="const", bufs=1))
        persist = es.enter_context(tc.tile_pool(name="persist", bufs=1))
        small = es.enter_context(tc.tile_pool(name="small", bufs=3))
        dram = es.enter_context(tc.tile_pool(name="dram", bufs=2, space="DRAM"))

        ident = const.tile([128, 128], F32)
        make_identity(nc, ident[:])
        eps_t = const.tile([128, 1], F32)
        nc.vector.memset(eps_t[:], EPS)
        if fc_bias_nonzero:
            ones1 = const.tile([1, 128], BF16)
            nc.vector.memset(ones1[:], 1.0)

        x = [persist.tile([128, S * L + 2], F32R, tag=f"x{c}", name=f"x{c}")
             for c in range(CH)]
        for c in range(CH):
            nc.vector.memset(x[c][:, S * L:].bitcast(F32), 0.0)
        xfc = [persist.tile([128, S * L], BF16, tag=f"xfc{c}", name=f"xfc{c}")
               for c in range(CH)]

        # ================= Phase A: toInit MLP, embedding gather, init conv
        with ExitStack() as esA:
            initp = esA.enter_context(tc.tile_pool(name="initp", bufs=1))
            psT = esA.enter_context(tc.tile_pool(name="psT", bufs=2, space="PSUM"))
            psI = esA.enter_context(tc.tile_pool(name="psI", bufs=2, space="PSUM"))
            psE = esA.enter_context(tc.tile_pool(name="psE", bufs=2, space="PSUM"))

            tw1 = initp.tile([128, EMB], F32R, tag="tw1")
            nc.sync.dma_start(out=tw1[:], in_=ti_w1T[:])
            tw2 = []
            for c in range(CH):
                t_ = initp.tile([128, EMB], F32R, tag=f"tw2_{c}")
                nc.sync.dma_start(out=t_[:], in_=ti_w2T[c * 128:(c + 1) * 128, :])
                tw2.append(t_)
            w1i, w2i, w12 = [], [], []
            for name, src, dst in (("w1i", w1iT, w1i), ("w2i", w2iT, w2i),
                                   ("w12", w12T, w12)):
                for c in range(CH):
                    t_ = initp.tile([128, NF], F32R, tag=f"{name}_{c}")
                    nc.sync.dma_start(out=t_[:], in_=src[c * 128:(c + 1) * 128, :])
                    dst.append(t_)
            b1t = initp.tile([128, CH], F32, tag="b1t")
            nc.sync.dma_start(out=b1t[:], in_=chvec(ti_b1))
            b2t = initp.tile([128, CH], F32, tag="b2t")
            nc.sync.dma_start(out=b2t[:], in_=chvec(ti_b2))
            ibt = initp.tile([128, CH], F32, tag="ibt")
            nc.sync.dma_start(out=ibt[:], in_=chvec(init_b))

            latb = initp.tile([128, S], F32R, tag="latb")
            nc.sync.dma_start(out=latb[:], in_=latT[:])

            # toInit: h = (relu(lat @ w1.T + b1)) @ w2.T + b2   -> [ch, sample]
            h1 = []
            for e in range(CH):
                p_ = psI.tile([128, S], F32, tag="ti", space="PSUM")
                nc.tensor.matmul(p_[:], tw1[:, e * 128:(e + 1) * 128], latb[:],
                                 start=True, stop=True)
                t_ = initp.tile([128, S], F32R, tag=f"h1_{e}")
                nc.scalar.activation(out=t_[:], in_=p_[:], func=RELU,
                                     bias=b1t[:, e:e + 1])
                h1.append(t_)
            h = []
            for e2 in range(CH):
                p_ = psI.tile([128, S], F32, tag="ti", space="PSUM")
                for e in range(CH):
                    nc.tensor.matmul(p_[:], tw2[e][:, e2 * 128:(e2 + 1) * 128],
                                     h1[e][:], start=(e == 0), stop=(e == CH - 1))
                t_ = initp.tile([128, S], F32R, tag=f"h_{e2}")
                nc.scalar.activation(out=t_[:], in_=p_[:], func=IDENT,
                                     bias=b2t[:, e2:e2 + 1])
                h.append(t_)
            v, x0 = [], []
            for c in range(CH):
                p_ = psI.tile([128, S], F32, tag="ti", space="PSUM")
                for ci in range(CH):
                    nc.tensor.matmul(p_[:], w2i[ci][:, c * 128:(c + 1) * 128],
                                     h[ci][:], start=(ci == 0), stop=(ci == CH - 1))
                t_ = initp.tile([128, S], F32, tag=f"v_{c}")
                nc.scalar.activation(out=t_[:], in_=p_[:], func=IDENT,
                                     bias=ibt[:, c:c + 1])
                v.append(t_)
                p2 = psI.tile([128, S], F32, tag="ti", space="PSUM")
                for ci in range(CH):
                    nc.tensor.matmul(p2[:], w12[ci][:, c * 128:(c + 1) * 128],
                                     h[ci][:], start=(ci == 0), stop=(ci == CH - 1))
                t2 = initp.tile([128, S], F32, tag=f"x0_{c}")
                nc.scalar.activation(out=t2[:], in_=p2[:], func=IDENT,
                                     bias=ibt[:, c:c + 1])
                x0.append(t2)

            # embedding gather + transpose to [ci, pos]
            embT = [initp.tile([128, S * T], F32R, tag=f"embT{c}", name=f"embT{c}")
                    for c in range(CH)]
            for g in range(S * T // 128):
                idx_sb = small.tile([128, 1], I32, tag="idx")
                nc.sync.dma_start(out=idx_sb[:], in_=helper[g * 128:(g + 1) * 128, :])
                ge = small.tile([128, EMB], F32, tag="gemb")
                nc.gpsimd.indirect_dma_start(
                    out=ge[:], out_offset=None, in_=emb[:],
                    in_offset=bass.IndirectOffsetOnAxis(ap=idx_sb[:, :1], axis=0))
                for c in range(CH):
                    tp = psT.tile([128, 128], F32, tag="tp", space="PSUM")
                    nc.tensor.transpose(tp[:], ge[:, c * 128:(c + 1) * 128], ident[:])
                    nc.vector.tensor_copy(out=embT[c][:, g * 128:(g + 1) * 128],
                                          in_=tp[:])

            # init conv: x[:, l>=1] = W1i @ embT + v ; x[:, 0] = x0
            for c in range(CH):
                for s in range(S):
                    pe1 = psE.tile([128, T], F32, tag="e1", space="PSUM")
                    for ci in range(CH):
                        nc.tensor.matmul(pe1[:], w1i[ci][:, c * 128:(c + 1) * 128],
                                         embT[ci][:, s * T:(s + 1) * T],
                                         start=(ci == 0), stop=(ci == CH - 1))
                    nc.scalar.activation(out=x[c][:, s * L + 1:(s + 1) * L],
                                         in_=pe1[:], func=IDENT,
                                         bias=v[c][:, s:s + 1])
                    nc.vector.tensor_copy(out=x[c][:, s * L:s * L + 1],
                                          in_=x0[c][:, s:s + 1])

        if dump == "init":
            for c in range(CH):
                nc.sync.dma_start(out=dbg[c * 128:(c + 1) * 128, :],
                                  in_=x[c][:, :S * L].bitcast(F32))

        # ================= Phase B: 10 residual blocks
        with ExitStack() as esB:
            wt = esB.enter_context(tc.tile_pool(name="wt", bufs=1))
            ybp = esB.enter_context(tc.tile_pool(name="ybp", bufs=2))
            zp = esB.enter_context(tc.tile_pool(name="zp", bufs=1))
            psY = esB.enter_context(tc.tile_pool(name="psY", bufs=6, space="PSUM"))

            def conv(mats, rhs_tiles, rhs_stride, tap_d, Lout):
                """mats: list of weight-mats (each list of CH [128,NF] tiles).
                Returns (yb tiles bf16 [128, S*Lout], mv [128, CH, 2])."""
                Lpad = Lout + (Lout % 2)
                yb = [ybp.tile([128, S * Lout], F32R, tag=f"yb{c}", name=f"yb{c}")
                      for c in range(CH)]
                st = small.tile([128, CH, S, 6], F32, tag="st")
                nmm = len(mats) * CH
                for c in range(CH):
                    for s in range(S):
                        y_ps = psY.tile([128, Lpad], F32, tag="y", space="PSUM")
                        i = 0
                        for t_i, mat in enumerate(mats):
                            off = s * rhs_stride + t_i * tap_d
                            for ci in range(CH):
                                nc.tensor.matmul(
                                    y_ps[:], mat[ci][:, c * 128:(c + 1) * 128],
                                    rhs_tiles[ci][:, off:off + Lpad],
                                    start=(i == 0), stop=(i == nmm - 1))
                                i += 1
                        if s % 2 == 0:
                            nc.vector.tensor_copy(
                                out=yb[c][:, s * Lout:(s + 1) * Lout],
                                in_=y_ps[:, :Lout])
                        else:
                            nc.scalar.copy(
                                out=yb[c][:, s * Lout:(s + 1) * Lout],
                                in_=y_ps[:, :Lout])
                        nc.vector.bn_stats(out=st[:, c, s, :],
                                           in_=yb[c][:, s * Lout:(s + 1) * Lout])
                mv = small.tile([128, CH, 2], F32, tag="mv")
                for c in range(CH):
                    nc.vector.bn_aggr(out=mv[:, c, :], in_=st[:, c])
                return yb, mv

            def bn_sync(mv, n_loc, g_ap, b_ap):
                """Global (8-core) BN stats from local (mean, var); returns
                per-channel scale/shift tiles s,t: z = relu(s*y + t)."""
                NN = float(N_CORES * n_loc)
                part = small.tile([128, 2 * CH], F32, tag="part")
                nc.vector.tensor_scalar_mul(out=part[:, 0:CH], in0=mv[:, :, 0],
                                            scalar1=float(n_loc))
                tmp = small.tile([128, CH], F32, tag="tmp")
                nc.vector.tensor_tensor(out=tmp[:], in0=mv[:, :, 0],
                                        in1=mv[:, :, 0], op=MULT)
                nc.vector.tensor_tensor(out=part[:, CH:2 * CH], in0=mv[:, :, 1],
                                        in1=tmp[:], op=ADD)
                nc.vector.tensor_scalar_mul(out=part[:, CH:2 * CH],
                                            in0=part[:, CH:2 * CH],
                                            scalar1=float(n_loc))
                d_in = dram.tile([128, 2 * CH], F32, tag="agin")
                d_out = dram.tile([N_CORES * 128, 2 * CH], F32, tag="agout")
                nc.sync.dma_start(out=d_in[:], in_=part[:])
                nc.gpsimd.collective_compute(
                    "AllGather", mybir.AluOpType.bypass,
                    replica_groups=[list(range(N_CORES))],
                    ins=[d_in.opt()], outs=[d_out.opt()])
                gath = small.tile([128, N_CORES, 2 * CH], F32, tag="gath")
                nc.sync.dma_start(out=gath[:],
                                  in_=d_out[:].rearrange("(r p) c -> p r c", p=128))
                nc.vector.tensor_tensor(out=gath[:, 0:4, :], in0=gath[:, 0:4, :],
                                        in1=gath[:, 4:8, :], op=ADD)
                nc.vector.tensor_tensor(out=gath[:, 0:2, :], in0=gath[:, 0:2, :],
                                        in1=gath[:, 2:4, :], op=ADD)
                G = small.tile([128, 2 * CH], F32, tag="G")
                nc.vector.tensor_tensor(out=G[:], in0=gath[:, 0, :],
                                        in1=gath[:, 1, :], op=ADD)
                mean = small.tile([128, CH], F32, tag="mean")
                nc.vector.tensor_scalar_mul(out=mean[:], in0=G[:, 0:CH],
                                            scalar1=1.0 / NN)
                var = small.tile([128, CH], F32, tag="var")
                nc.vector.tensor_scalar_mul(out=var[:], in0=G[:, CH:2 * CH],
                                            scalar1=1.0 / NN)
                m2 = small.tile([128, CH], F32, tag="m2")
                nc.vector.tensor_tensor(out=m2[:], in0=mean[:], in1=mean[:], op=MULT)
                nc.vector.tensor_tensor(out=var[:], in0=var[:], in1=m2[:], op=SUB)
                sd = small.tile([128, CH], F32, tag="sd")
                nc.scalar.activation(out=sd[:], in_=var[:], func=SQRT,
                                     bias=eps_t[:])
                rinv = small.tile([128, CH], F32, tag="rinv")
                nc.vector.reciprocal(out=rinv[:], in_=sd[:])
                gt = small.tile([128, CH], F32, tag="gt")
                nc.sync.dma_start(out=gt[:], in_=g_ap)
                bt = small.tile([128, CH], F32, tag="bt")
                nc.sync.dma_start(out=bt[:], in_=b_ap)
                s_t = small.tile([128, CH], F32, tag="s_t")
                nc.vector.tensor_tensor(out=s_t[:], in0=gt[:], in1=rinv[:], op=MULT)
                t_t = small.tile([128, CH], F32, tag="t_t")
                nc.vector.tensor_tensor(out=t_t[:], in0=mean[:], in1=s_t[:], op=MULT)
                nc.vector.tensor_tensor(out=t_t[:], in0=bt[:], in1=t_t[:], op=SUB)
                return s_t, t_t

            for k, d in enumerate(DILS):
                L2 = L + 2 * d
                stride = L + 4 * d
                # weights for this block (bufs=1: reload overlaps prior reads)
                wmats = []
                for m in range(5):
                    mats_ci = []
                    for ci in range(CH):
                        wt_t = wt.tile([128, NF], F32R, tag=f"w{m}_{ci}")
                        nc.sync.dma_start(
                            out=wt_t[:],
                            in_=wconv[k, m, ci * 128:(ci + 1) * 128, :])
                        mats_ci.append(wt_t)
                    wmats.append(mats_ci)

                # ---- c1 (1x1) on xb
                yb1, mv1 = conv([wmats[0]], x, L, 0, L)
                s1, t1 = bn_sync(mv1, S * L, chvec(bng[k, 0]), chvec(bnb[k, 0]))
                z1p = [zp.tile([128, S * stride + 2], F32R, tag=f"z1p{c}", name=f"z1p{c}")
                       for c in range(CH)]
                for c in range(CH):
                    nc.vector.memset(z1p[c][:].bitcast(F32), 0.0)
                    for s in range(S):
                        o0 = s * stride + 2 * d
                        nc.scalar.activation(out=z1p[c][:, o0:o0 + L],
                                             in_=yb1[c][:, s * L:(s + 1) * L],
                                             func=RELU, scale=s1[:, c:c + 1],
                                             bias=t1[:, c:c + 1])

                # ---- c2 (k=3 dilated) on z1p
                yb2, mv2 = conv(wmats[1:4], z1p, stride, d, L2)
                s2, t2 = bn_sync(mv2, S * L2, chvec(bng[k, 1]), chvec(bnb[k, 1]))
                z2 = [zp.tile([128, S * L2 + 2], F32R, tag=f"z2_{c}", name=f"z2_{c}")
                      for c in range(CH)]
                for c in range(CH):
                    nc.vector.memset(z2[c][:, S * L2:].bitcast(F32), 0.0)
                    for s in range(S):
                        nc.scalar.activation(out=z2[c][:, s * L2:(s + 1) * L2],
                                             in_=yb2[c][:, s * L2:(s + 1) * L2],
                                             func=RELU, scale=s2[:, c:c + 1],
                                             bias=t2[:, c:c + 1])

                # ---- c3 (1x1) on z2
                yb3, mv3 = conv([wmats[4]], z2, L2, 0, L2)
                s3, t3 = bn_sync(mv3, S * L2, chvec(bng[k, 2]), chvec(bnb[k, 2]))
                # z3 trimmed to L, add residual, refresh bf16 mirror
                for c in range(CH):
                    for s in range(S):
                        zt = small.tile([128, L], F32, tag="zt")
                        nc.scalar.activation(out=zt[:],
                                             in_=yb3[c][:, s * L2:s * L2 + L],
                                             func=RELU, scale=s3[:, c:c + 1],
                                             bias=t3[:, c:c + 1])
                        nc.vector.tensor_tensor(out=x[c][:, s * L:(s + 1) * L],
                                                in0=x[c][:, s * L:(s + 1) * L],
                                                in1=zt[:], op=ADD)

                if dump == k:
                    for c in range(CH):
                        nc.sync.dma_start(out=dbg[c * 128:(c + 1) * 128, :],
                                          in_=x[c][:, :S * L].bitcast(F32))
            for c in range(CH):
                nc.vector.tensor_copy(out=xfc[c][:], in_=x[c][:, :S * L])

        # ================= Phase C: FC head out[pos, voc]
        with ExitStack() as esC:
            psF = esC.enter_context(tc.tile_pool(name="psF", bufs=6, space="PSUM"))
            fcp = esC.enter_context(tc.tile_pool(name="fcp", bufs=1))
            if fc_bias_nonzero:
                fcb_sb = fcp.tile([1, VOC], BF16, tag="fcb")
                nc.sync.dma_start(out=fcb_sb[:], in_=fc_b[:])
            STAGES = [3072, 3072, 3072, 2784]
            fcw = []
            for ci in range(CH):
                row = []
                vb = 0
                for si, sz in enumerate(STAGES):
                    t_ = fcp.tile([128, sz], BF16, tag=f"fcw{ci}_{si}",
                                  name=f"fcw{ci}_{si}")
                    nc.sync.dma_start(out=t_[:],
                                      in_=fc_wT[ci * 128:(ci + 1) * 128,
                                                vb:vb + sz])
                    row.append(t_)
                    vb += sz
                fcw.append(row)
            n_vch = (VOC + 511) // 512
            for s in range(S):
                for hh in range(2):
                    p0 = s * L + 1 + hh * 128
                    for vci in range(n_vch):
                        v0 = 512 * vci
                        vw = min(512, VOC - v0)
                        si = vci // 6
                        off = v0 - si * 3072
                        pf = psF.tile([128, 512], F32, tag="fcy", space="PSUM")
                        for ci in range(CH):
                            nc.tensor.matmul(
                                pf[:, :vw], xfc[ci][:, p0:p0 + 128],
                                fcw[ci][si][:, off:off + vw], start=(ci == 0),
                                stop=(ci == CH - 1 and not fc_bias_nonzero))
                        if fc_bias_nonzero:
                            nc.tensor.matmul(pf[:, :vw], ones1[:, :],
                                             fcb_sb[:, v0:v0 + vw],
                                             start=False, stop=True)
                        o_sb = small.tile([128, 512], F32, tag="osb")
                        nc.vector.tensor_copy(out=o_sb[:, :vw], in_=pf[:, :vw])
                        nc.sync.dma_start(
                            out=out[s, hh * 128:(hh + 1) * 128, v0:v0 + vw],
                            in_=o_sb[:, :vw])

    nc.compile()
    return nc


def _prep(helper, latent_vector, params):
    bf = ml_dtypes.bfloat16
    p = params
    helper = np.asarray(helper).astype(np.int32)
    lat = np.asarray(latent_vector, dtype=np.float32)

    def a32(v):
        return np.asarray(v, dtype=np.float32)

    def tb(v):  # transpose + bf16
        return np.ascontiguousarray(a32(v).T).astype(bf)

    def tf(v):  # transpose, f32
        return np.ascontiguousarray(a32(v).T)

    emb_b = a32(p["emb"])
    w1 = a32(p["init_w"])[:, :, 0]
    w1i, w2i = w1[:, :EMB], w1[:, EMB:]
    wconv = np.empty((10, 5, NF, NF), dtype=np.float32)
    bng = np.empty((10, 3, NF), dtype=np.float32)
    bnb = np.empty((10, 3, NF), dtype=np.float32)
    for k, blk in enumerate(p["blocks"]):
        wconv[k, 0] = tf(a32(blk["c1_w"])[:, :, 0])
        c2 = a32(blk["c2_w"])
        for t in range(3):
            wconv[k, 1 + t] = tf(c2[:, :, t])
        wconv[k, 4] = tf(a32(blk["c3_w"])[:, :, 0])
        for j, nm in enumerate(("bn1", "bn2", "bn3")):
            bng[k, j] = a32(blk[f"{nm}_g"])
            bnb[k, j] = a32(blk[f"{nm}_b"])
    fc_b = a32(p["fc_b"])
    shared = {
        "emb": emb_b,
        "ti_w1T": tf(p["ti_w1"]),
        "ti_w2T": tf(p["ti_w2"]),
        "ti_b1": a32(p["ti_b1"]),
        "ti_b2": a32(p["ti_b2"]),
        "w1iT": tf(w1i),
        "w2iT": tf(w2i),
        "w12T": tf(w1i + w2i),
        "init_b": a32(p["init_b"]),
        "wconv": wconv,
        "bng": bng,
        "bnb": bnb,
        "fc_wT": tb(p["fc_w"]),
        "fc_b": fc_b.reshape(1, VOC).astype(bf),
    }
    fc_nz = bool(np.any(fc_b != 0))
    in_maps = []
    for c in range(N_CORES):
        m = dict(shared)
        m["helper"] = np.ascontiguousarray(
            helper[c * S:(c + 1) * S].reshape(S * T, 1))
        m["latT"] = np.ascontiguousarray(lat[c * S:(c + 1) * S].T)
        in_maps.append(m)
    return in_maps, fc_nz


def _get(fc_nz, dump=None):
    key = (fc_nz, dump)
    if key not in _cache:
        _cache[key] = _build(fc_nz, dump)
    return _cache[key]


def _run(inputs, dump=None, trace=False):
    in_maps, fc_nz = _prep(**inputs)
    nc = _get(fc_nz, dump)
    res = run_bass_kernel_spmd(nc, in_maps, core_ids=list(range(N_CORES)),
                               trace=trace)
    out = np.concatenate([res.results[c]["out"] for c in range(N_CORES)], axis=0)
    extras = {}
    if dump is not None:
        extras["dbg"] = [res.results[c]["dbg"] for c in range(N_CORES)]
    return out, extras, res


def kernel(helper, latent_vector, params):
    out, _, _ = _run({"helper": helper, "latent_vector": latent_vector,
                      "params": params})
    return out


# revision 7
# speedup vs baseline: 2.8432x; 2.8432x over previous
"""Trainium2 Bass kernel for nn_Decoder (dense CNN decoder, 8 NeuronCores).

Sharding: data-parallel over batch (B=32 -> 4 samples/core). BatchNorm batch
stats are synced exactly via small AllGather collectives (sum/sumsq per
channel). Convs run as bf16 TensorE matmuls with f32 PSUM accumulation;
dilated k=3 convs are 3 shifted 1x1 matmuls accumulated in PSUM. All weight
transposes (lhsT layouts, fc_w.T) are prepared host-side. Conv biases feeding
BatchNorm cancel exactly (BN shift invariance) and are dropped.

Self-contained: hardcodes all shapes; reads nothing from /root/problem.
"""
import sys

sys.path.insert(0, "/opt/trn_rl_repo")

from contextlib import ExitStack

import numpy as np
import ml_dtypes

import concourse.bass as bass
import concourse.tile as tile
from concourse import bacc, mybir
from concourse.bass_utils import run_bass_kernel_spmd
from concourse.masks import make_identity

N_CORES = 8
B = 32
S = B // N_CORES          # samples per core
T = 256
L = T + 1                 # 257 positions incl. init token
VOC = 12000
EMB = 512
NF = 512
LATENT = 128
DILS = [1, 2, 4, 8, 16, 1, 2, 4, 8, 16]
EPS = 1e-5
CH = NF // 128            # 4 channel chunks

BF16 = mybir.dt.bfloat16
F32 = mybir.dt.float32
F32R = mybir.dt.float32r
I32 = mybir.dt.int32
RELU = mybir.ActivationFunctionType.Relu
IDENT = mybir.ActivationFunctionType.Identity
SQRT = mybir.ActivationFunctionType.Sqrt
ADD = mybir.AluOpType.add
SUB = mybir.AluOpType.subtract
MULT = mybir.AluOpType.mult

_cache = {}


def _build(fc_bias_nonzero, dump=None):
    """dump: None | 'init' | int k (dump x after block k) -> extra output 'dbg'."""
    nc = bacc.Bacc("TRN2", target_bir_lowering=False, debug=False,
                   num_devices=N_CORES)

    def inp(name, shape, dt):
        return nc.dram_tensor(name, shape, dt, kind="ExternalInput").ap()

    helper = inp("helper", [S * T, 1], I32)
    latT = inp("latT", [LATENT, S], F32R)
    emb = inp("emb", [VOC, EMB], F32)
    ti_w1T = inp("ti_w1T", [LATENT, EMB], F32R)
    ti_w2T = inp("ti_w2T", [EMB, EMB], F32R)
    ti_b1 = inp("ti_b1", [EMB], F32)
    ti_b2 = inp("ti_b2", [EMB], F32)
    w1iT = inp("w1iT", [EMB, NF], F32R)
    w2iT = inp("w2iT", [EMB, NF], F32R)
    w12T = inp("w12T", [EMB, NF], F32R)
    init_b = inp("init_b", [NF], F32)
    wconv = inp("wconv", [10, 5, NF, NF], F32R)
    bng = inp("bng", [10, 3, NF], F32)
    bnb = inp("bnb", [10, 3, NF], F32)
    fc_wT = inp("fc_wT", [NF, VOC], BF16)
    fc_b = inp("fc_b", [1, VOC], BF16)
    out = nc.dram_tensor("out", [S, T, VOC], F32, kind="ExternalOutput").ap()
    dbg = None
    if dump is not None:
        dbg = nc.dram_tensor("dbg", [NF, S * L], F32, kind="ExternalOutput").ap()

    def chvec(ap1d):
        # [512] dram vector -> [128, 4] AP (partition p, chunk c) = v[c*128+p]
        return ap1d.rearrange("(c p) -> p c", p=128)

    with tile.TileContext(nc) as tc, ExitStack() as es:
        const = es.enter_context(tc.tile_pool(name="const", bufs=1))
        persist = es.enter_context(tc.tile_pool(name="persist", bufs=1))
        small = es.enter_context(tc.tile_pool(name="small", bufs=3))
        dram = es.enter_context(tc.tile_pool(name="dram", bufs=2, space="DRAM"))

        ident = const.tile([128, 128], F32)
        make_identity(nc, ident[:])
        eps_t = const.tile([128, 1], F32)
        nc.vector.memset(eps_t[:], EPS)
        if fc_bias_nonzero:
            ones1 = const.tile([1, 128], BF16)
            nc.vector.memset(ones1[:], 1.0)

        x = [persist.tile([128, S * L + 2], F32R, tag=f"x{c}", name=f"x{c}")
             for c in range(CH)]
        for c in range(CH):
            nc.vector.memset(x[c][:, S * L:].bitcast(F32), 0.0)
        xfc = [persist.tile([128, S * L], BF16, tag=f"xfc{c}", name=f"xfc{c}")
               for c in range(CH)]

        # ================= Phase A: toInit MLP, embedding gather, init conv
        with ExitStack() as esA:
            initp = esA.enter_context(tc.tile_pool(name="initp", bufs=1))
            psT = esA.enter_context(tc.tile_pool(name="psT", bufs=2, space="PSUM"))
            psI = esA.enter_context(tc.tile_pool(name="psI", bufs=2, space="PSUM"))
            psE = esA.enter_context(tc.tile_pool(name="psE", bufs=2, space="PSUM"))

            tw1 = initp.tile([128, EMB], F32R, tag="tw1")
            nc.sync.dma_start(out=tw1[:], in_=ti_w1T[:])
            tw2 = []
            for c in range(CH):
                t_ = initp.tile([128, EMB], F32R, tag=f"tw2_{c}")
                nc.sync.dma_start(out=t_[:], in_=ti_w2T[c * 128:(c + 1) * 128, :])
                tw2.append(t_)
            w1i, w2i, w12 = [], [], []
            for name, src, dst in (("w1i", w1iT, w1i), ("w2i", w2iT, w2i),
                                   ("w12", w12T, w12)):
                for c in range(CH):
                    t_ = initp.tile([128, NF], F32R, tag=f"{name}_{c}")
                    nc.sync.dma_start(out=t_[:], in_=src[c * 128:(c + 1) * 128, :])
                    dst.append(t_)
            b1t = initp.tile([128, CH], F32, tag="b1t")
            nc.sync.dma_start(out=b1t[:], in_=chvec(ti_b1))
            b2t = initp.tile([128, CH], F32, tag="b2t")
            nc.sync.dma_start(out=b2t[:], in_=chvec(ti_b2))
            ibt = initp.tile([128, CH], F32, tag="ibt")
            nc.sync.dma_start(out=ibt[:], in_=chvec(init_b))

            latb = initp.tile([128, S], F32R, tag="latb")
            nc.sync.dma_start(out=latb[:], in_=latT[:])

            # toInit: h = (relu(lat @ w1.T + b1)) @ w2.T + b2   -> [ch, sample]
            h1 = []
            for e in range(CH):
                p_ = psI.tile([128, S], F32, tag="ti", space="PSUM")
                nc.tensor.matmul(p_[:], tw1[:, e * 128:(e + 1) * 128], latb[:],
                                 start=True, stop=True)
                t_ = initp.tile([128, S], F32R, tag=f"h1_{e}")
                nc.scalar.activation(out=t_[:], in_=p_[:], func=RELU,
                                     bias=b1t[:, e:e + 1])
                h1.append(t_)
            h = []
            for e2 in range(CH):
                p_ = psI.tile([128, S], F32, tag="ti", space="PSUM")
                for e in range(CH):
                    nc.tensor.matmul(p_[:], tw2[e][:, e2 * 128:(e2 + 1) * 128],
                                     h1[e][:], start=(e == 0), stop=(e == CH - 1))
                t_ = initp.tile([128, S], F32R, tag=f"h_{e2}")
                nc.scalar.activation(out=t_[:], in_=p_[:], func=IDENT,
                                     bias=b2t[:, e2:e2 + 1])
                h.append(t_)
            v, x0 = [], []
            for c in range(CH):
                p_ = psI.tile([128, S], F32, tag="ti", space="PSUM")
                for ci in range(CH):
                    nc.tensor.matmul(p_[:], w2i[ci][:, c * 128:(c + 1) * 128],
                                     h[ci][:], start=(ci == 0), stop=(ci == CH - 1))
                t_ = initp.tile([128, S], F32, tag=f"v_{c}")
                nc.scalar.activation(out=t_[:], in_=p_[:], func=IDENT,
                                     bias=ibt[:, c:c + 1])
                v.append(t_)
                p2 = psI.tile([128, S], F32, tag="ti", space="PSUM")
                for ci in range(CH):
                    nc.tensor.matmul(p2[:], w12[ci][:, c * 128:(c + 1) * 128],
                                     h[ci][:], start=(ci == 0), stop=(ci == CH - 1))
                t2 = initp.tile([128, S], F32, tag=f"x0_{c}")
                nc.scalar.activation(out=t2[:], in_=p2[:], func=IDENT,
                                     bias=ibt[:, c:c + 1])
                x0.append(t2)

            # embedding gather + transpose to [ci, pos]
            embT = [initp.tile([128, S * T], F32R, tag=f"embT{c}", name=f"embT{c}")
                    for c in range(CH)]
            for g in range(S * T // 128):
                idx_sb = small.tile([128, 1], I32, tag="idx")
                nc.sync.dma_start(out=idx_sb[:], in_=helper[g * 128:(g + 1) * 128, :])
                ge = small.tile([128, EMB], F32, tag="gemb")
                nc.gpsimd.indirect_dma_start(
                    out=ge[:], out_offset=None, in_=emb[:],
                    in_offset=bass.IndirectOffsetOnAxis(ap=idx_sb[:, :1], axis=0))
                for c in range(CH):
                    tp = psT.tile([128, 128], F32, tag="tp", space="PSUM")
                    nc.tensor.transpose(tp[:], ge[:, c * 128:(c + 1) * 128], ident[:])
                    nc.vector.tensor_copy(out=embT[c][:, g * 128:(g + 1) * 128],
                                          in_=tp[:])

            # init conv: x[:, l>=1] = W1i @ embT + v ; x[:, 0] = x0
            for c in range(CH):
                for s in range(S):
                    pe1 = psE.tile([128, T], F32, tag="e1", space="PSUM")
                    for ci in range(CH):
                        nc.tensor.matmul(pe1[:], w1i[ci][:, c * 128:(c + 1) * 128],
                                         embT[ci][:, s * T:(s + 1) * T],
                                         start=(ci == 0), stop=(ci == CH - 1))
                    nc.scalar.activation(out=x[c][:, s * L + 1:(s + 1) * L],
                                         in_=pe1[:], func=IDENT,
                                         bias=v[c][:, s:s + 1])
                    nc.vector.tensor_copy(out=x[c][:, s * L:s * L + 1],
                                          in_=x0[c][:, s:s + 1])

        if dump == "init":
            for c in range(CH):
                nc.sync.dma_start(out=dbg[c * 128:(c + 1) * 128, :],
                                  in_=x[c][:, :S * L].bitcast(F32))

        # ================= Phase B: 10 residual blocks
        with ExitStack() as esB:
            wt = esB.enter_context(tc.tile_pool(name="wt", bufs=2))
            ybp = esB.enter_context(tc.tile_pool(name="ybp", bufs=2))
            zp = esB.enter_context(tc.tile_pool(name="zp", bufs=1))
            psY = esB.enter_context(tc.tile_pool(name="psY", bufs=7, space="PSUM"))

            def conv(mats, rhs_tiles, rhs_stride, tap_d, Lout):
                """mats: list of weight-mats (each list of CH [128,NF] tiles).
                Returns (yb tiles bf16 [128, S*Lout], mv [128, CH, 2])."""
                Lpad = Lout + (Lout % 2)
                yb = [ybp.tile([128, S * Lout], F32R, tag=f"yb{c}", name=f"yb{c}")
                      for c in range(CH)]
                st = small.tile([128, CH, S, 6], F32, tag="st")
                nmm = len(mats) * CH
                for c in range(CH):
                    for s in range(S):
                        y_ps = psY.tile([128, Lpad], F32, tag="y", space="PSUM")
                        i = 0
                        for t_i, mat in enumerate(mats):
                            off = s * rhs_stride + t_i * tap_d
                            for ci in range(CH):
                                nc.tensor.matmul(
                                    y_ps[:], mat[ci][:, c * 128:(c + 1) * 128],
                                    rhs_tiles[ci][:, off:off + Lpad],
                                    start=(i == 0), stop=(i == nmm - 1))
                                i += 1
                        if s % 2 == 0:
                            nc.vector.tensor_copy(
                                out=yb[c][:, s * Lout:(s + 1) * Lout],
                                in_=y_ps[:, :Lout])
                        else:
                            nc.scalar.copy(
                                out=yb[c][:, s * Lout:(s + 1) * Lout],
                                in_=y_ps[:, :Lout])
                        nc.vector.bn_stats(out=st[:, c, s, :],
                                           in_=yb[c][:, s * Lout:(s + 1) * Lout])
                mv = small.tile([128, CH, 2], F32, tag="mv")
                for c in range(CH):
                    nc.vector.bn_aggr(out=mv[:, c, :], in_=st[:, c])
                return yb, mv

            def bn_sync(mv, n_loc, g_ap, b_ap):
                """Global (8-core) BN stats from local (mean, var); returns
                per-channel scale/shift tiles s,t: z = relu(s*y + t)."""
                NN = float(N_CORES * n_loc)
                part = small.tile([128, 2 * CH], F32, tag="part")
                nc.vector.tensor_scalar_mul(out=part[:, 0:CH], in0=mv[:, :, 0],
                                            scalar1=float(n_loc))
                tmp = small.tile([128, CH], F32, tag="tmp")
                nc.vector.tensor_tensor(out=tmp[:], in0=mv[:, :, 0],
                                        in1=mv[:, :, 0], op=MULT)
                nc.vector.tensor_tensor(out=part[:, CH:2 * CH], in0=mv[:, :, 1],
                                        in1=tmp[:], op=ADD)
                nc.vector.tensor_scalar_mul(out=part[:, CH:2 * CH],
                                            in0=part[:, CH:2 * CH],
                                            scalar1=float(n_loc))
                d_in = dram.tile([128, 2 * CH], F32, tag="agin")
                d_out = dram.tile([N_CORES * 128, 2 * CH], F32, tag="agout")
                nc.sync.dma_start(out=d_in[:], in_=part[:])
                nc.gpsimd.collective_compute(
                    "AllGather", mybir.AluOpType.bypass,
                    replica_groups=[list(range(N_CORES))],
                    ins=[d_in.opt()], outs=[d_out.opt()])
                gath = small.tile([128, N_CORES, 2 * CH], F32, tag="gath")
                nc.sync.dma_start(out=gath[:],
                                  in_=d_out[:].rearrange("(r p) c -> p r c", p=128))
                nc.vector.tensor_tensor(out=gath[:, 0:4, :], in0=gath[:, 0:4, :],
                                        in1=gath[:, 4:8, :], op=ADD)
                nc.vector.tensor_tensor(out=gath[:, 0:2, :], in0=gath[:, 0:2, :],
                                        in1=gath[:, 2:4, :], op=ADD)
                G = small.tile([128, 2 * CH], F32, tag="G")
                nc.vector.tensor_tensor(out=G[:], in0=gath[:, 0, :],
                                        in1=gath[:, 1, :], op=ADD)
                mean = small.tile([128, CH], F32, tag="mean")
                nc.vector.tensor_scalar_mul(out=mean[:], in0=G[:, 0:CH],
                                            scalar1=1.0 / NN)
                var = small.tile([128, CH], F32, tag="var")
                nc.vector.tensor_scalar_mul(out=var[:], in0=G[:, CH:2 * CH],
                                            scalar1=1.0 / NN)
                m2 = small.tile([128, CH], F32, tag="m2")
                nc.vector.tensor_tensor(out=m2[:], in0=mean[:], in1=mean[:], op=MULT)
                nc.vector.tensor_tensor(out=var[:], in0=var[:], in1=m2[:], op=SUB)
                sd = small.tile([128, CH], F32, tag="sd")
                nc.scalar.activation(out=sd[:], in_=var[:], func=SQRT,
                                     bias=eps_t[:])
                rinv = small.tile([128, CH], F32, tag="rinv")
                nc.vector.reciprocal(out=rinv[:], in_=sd[:])
                gt = small.tile([128, CH], F32, tag="gt")
                nc.sync.dma_start(out=gt[:], in_=g_ap)
                bt = small.tile([128, CH], F32, tag="bt")
                nc.sync.dma_start(out=bt[:], in_=b_ap)
                s_t = small.tile([128, CH], F32, tag="s_t")
                nc.vector.tensor_tensor(out=s_t[:], in0=gt[:], in1=rinv[:], op=MULT)
                t_t = small.tile([128, CH], F32, tag="t_t")
                nc.vector.tensor_tensor(out=t_t[:], in0=mean[:], in1=s_t[:], op=MULT)
                nc.vector.tensor_tensor(out=t_t[:], in0=bt[:], in1=t_t[:], op=SUB)
                return s_t, t_t

            for k, d in enumerate(DILS):
                L2 = L + 2 * d
                stride = L + 4 * d
                # weights for this block (bufs=1: reload overlaps prior reads)
                wmats = []
                for m in range(5):
                    mats_ci = []
                    for ci in range(CH):
                        wt_t = wt.tile([128, NF], F32R, tag=f"w{m}_{ci}")
                        nc.sync.dma_start(
                            out=wt_t[:],
                            in_=wconv[k, m, ci * 128:(ci + 1) * 128, :])
                        mats_ci.append(wt_t)
                    wmats.append(mats_ci)

                # ---- c1 (1x1) on xb
                yb1, mv1 = conv([wmats[0]], x, L, 0, L)
                s1, t1 = bn_sync(mv1, S * L, chvec(bng[k, 0]), chvec(bnb[k, 0]))
                z1p = [zp.tile([128, S * stride + 2], F32R, tag=f"z1p{c}", name=f"z1p{c}")
                       for c in range(CH)]
                for c in range(CH):
                    nc.vector.memset(z1p[c][:].bitcast(F32), 0.0)
                    for s in range(S):
                        o0 = s * stride + 2 * d
                        nc.scalar.activation(out=z1p[c][:, o0:o0 + L],
                                             in_=yb1[c][:, s * L:(s + 1) * L],
                                             func=RELU, scale=s1[:, c:c + 1],
                                             bias=t1[:, c:c + 1])

                # ---- c2 (k=3 dilated) on z1p
                yb2, mv2 = conv(wmats[1:4], z1p, stride, d, L2)
                s2, t2 = bn_sync(mv2, S * L2, chvec(bng[k, 1]), chvec(bnb[k, 1]))
                z2 = [zp.tile([128, S * L2 + 2], F32R, tag=f"z2_{c}", name=f"z2_{c}")
                      for c in range(CH)]
                for c in range(CH):
                    nc.vector.memset(z2[c][:, S * L2:].bitcast(F32), 0.0)
                    for s in range(S):
                        nc.scalar.activation(out=z2[c][:, s * L2:(s + 1) * L2],
                                             in_=yb2[c][:, s * L2:(s + 1) * L2],
                                             func=RELU, scale=s2[:, c:c + 1],
                                             bias=t2[:, c:c + 1])

                # ---- c3 (1x1) on z2
                yb3, mv3 = conv([wmats[4]], z2, L2, 0, L2)
                s3, t3 = bn_sync(mv3, S * L2, chvec(bng[k, 2]), chvec(bnb[k, 2]))
                # z3 trimmed to L, add residual, refresh bf16 mirror
                for c in range(CH):
                    for s in range(S):
                        zt = small.tile([128, L], F32, tag="zt")
                        nc.scalar.activation(out=zt[:],
                                             in_=yb3[c][:, s * L2:s * L2 + L],
                                             func=RELU, scale=s3[:, c:c + 1],
                                             bias=t3[:, c:c + 1])
                        nc.vector.tensor_tensor(out=x[c][:, s * L:(s + 1) * L],
                                                in0=x[c][:, s * L:(s + 1) * L],
                                                in1=zt[:], op=ADD)

                if dump == k:
                    for c in range(CH):
                        nc.sync.dma_start(out=dbg[c * 128:(c + 1) * 128, :],
                                          in_=x[c][:, :S * L].bitcast(F32))
            for c in range(CH):
                nc.vector.tensor_copy(out=xfc[c][:], in_=x[c][:, :S * L])

        # ================= Phase C: FC head out[pos, voc]
        with ExitStack() as esC:
            psF = esC.enter_context(tc.tile_pool(name="psF", bufs=8, space="PSUM"))
            fcp = esC.enter_context(tc.tile_pool(name="fcp", bufs=1))
            if fc_bias_nonzero:
                fcb_sb = fcp.tile([1, VOC], BF16, tag="fcb")
                nc.sync.dma_start(out=fcb_sb[:], in_=fc_b[:])
            STAGES = [3072, 3072, 3072, 2784]
            fcw = []
            for ci in range(CH):
                row = []
                vb = 0
                for si, sz in enumerate(STAGES):
                    t_ = fcp.tile([128, sz], BF16, tag=f"fcw{ci}_{si}",
                                  name=f"fcw{ci}_{si}")
                    nc.sync.dma_start(out=t_[:],
                                      in_=fc_wT[ci * 128:(ci + 1) * 128,
                                                vb:vb + sz])
                    row.append(t_)
                    vb += sz
                fcw.append(row)
            n_vch = (VOC + 511) // 512
            for s in range(S):
                for hh in range(2):
                    p0 = s * L + 1 + hh * 128
                    for vci in range(n_vch):
                        v0 = 512 * vci
                        vw = min(512, VOC - v0)
                        si = vci // 6
                        off = v0 - si * 3072
                        pf = psF.tile([128, 512], F32, tag="fcy", space="PSUM")
                        for ci in range(CH):
                            nc.tensor.matmul(
                                pf[:, :vw], xfc[ci][:, p0:p0 + 128],
                                fcw[ci][si][:, off:off + vw], start=(ci == 0),
                                stop=(ci == CH - 1 and not fc_bias_nonzero))
                        if fc_bias_nonzero:
                            nc.tensor.matmul(pf[:, :vw], ones1[:, :],
                                             fcb_sb[:, v0:v0 + vw],
                                             start=False, stop=True)
                        o_sb = small.tile([128, 512], F32, tag="osb")
                        nc.vector.tensor_copy(out=o_sb[:, :vw], in_=pf[:, :vw])
                        nc.sync.dma_start(
                            out=out[s, hh * 128:(hh + 1) * 128, v0:v0 + vw],
                            in_=o_sb[:, :vw])

    nc.compile()
    return nc


def _prep(helper, latent_vector, params):
    bf = ml_dtypes.bfloat16
    p = params
    helper = np.asarray(helper).astype(np.int32)
    lat = np.asarray(latent_vector, dtype=np.float32)

    def a32(v):
        return np.asarray(v, dtype=np.float32)

    def tb(v):  # transpose + bf16
        return np.ascontiguousarray(a32(v).T).astype(bf)

    def tf(v):  # transpose, f32
        return np.ascontiguousarray(a32(v).T)

    emb_b = a32(p["emb"])
    w1 = a32(p["init_w"])[:, :, 0]
    w1i, w2i = w1[:, :EMB], w1[:, EMB:]
    wconv = np.empty((10, 5, NF, NF), dtype=np.float32)
    bng = np.empty((10, 3, NF), dtype=np.float32)
    bnb = np.empty((10, 3, NF), dtype=np.float32)
    for k, blk in enumerate(p["blocks"]):
        wconv[k, 0] = tf(a32(blk["c1_w"])[:, :, 0])
        c2 = a32(blk["c2_w"])
        for t in range(3):
            wconv[k, 1 + t] = tf(c2[:, :, t])
        wconv[k, 4] = tf(a32(blk["c3_w"])[:, :, 0])
        for j, nm in enumerate(("bn1", "bn2", "bn3")):
            bng[k, j] = a32(blk[f"{nm}_g"])
            bnb[k, j] = a32(blk[f"{nm}_b"])
    fc_b = a32(p["fc_b"])
    shared = {
        "emb": emb_b,
        "ti_w1T": tf(p["ti_w1"]),
        "ti_w2T": tf(p["ti_w2"]),
        "ti_b1": a32(p["ti_b1"]),
        "ti_b2": a32(p["ti_b2"]),
        "w1iT": tf(w1i),
        "w2iT": tf(w2i),
        "w12T": tf(w1i + w2i),
        "init_b": a32(p["init_b"]),
        "wconv": wconv,
        "bng": bng,
        "bnb": bnb,
        "fc_wT": tb(p["fc_w"]),
        "fc_b": fc_b.reshape(1, VOC).astype(bf),
    }
    fc_nz = bool(np.any(fc_b != 0))
    in_maps = []
    for c in range(N_CORES):
        m = dict(shared)
        m["helper"] = np.ascontiguousarray(
            helper[c * S:(c + 1) * S].reshape(S * T, 1))
        m["latT"] = np.ascontiguousarray(lat[c * S:(c + 1) * S].T)
        in_maps.append(m)
    return in_maps, fc_nz


def _get(fc_nz, dump=None):
    key = (fc_nz, dump)
    if key not in _cache:
        _cache[key] = _build(fc_nz, dump)
    return _cache[key]


def _run(inputs, dump=None, trace=False):
    in_maps, fc_nz = _prep(**inputs)
    nc = _get(fc_nz, dump)
    res = run_bass_kernel_spmd(nc, in_maps, core_ids=list(range(N_CORES)),
                               trace=trace)
    out = np.concatenate([res.results[c]["out"] for c in range(N_CORES)], axis=0)
    extras = {}
    if dump is not None:
        extras["dbg"] = [res.results[c]["dbg"] for c in range(N_CORES)]
    return out, extras, res


def kernel(helper, latent_vector, params):
    out, _, _ = _run({"helper": helper, "latent_vector": latent_vector,
                      "params": params})
    return out
